# revision 1
# baseline (speedup 1.0000x reference)
"""Trainium2 Bass kernel for 2D Gaussian Splatting (N=1024 gaussians, 256x256).

Math: sigma[p,i] is a quadratic polynomial in pixel coords, so
m1 = log(op_i) - sigma and m2 = log(op_i * col_i) - sigma are matmuls
F[128,6] @ G[6,*] with F the (constant) per-block pixel basis. Then
alpha = exp(m1), b = alpha*col = exp(m2) on the scalar engine,
beta = 1 - alpha on DVE, and front-to-back compositing is evaluated
back-to-front as one affine scan C = beta*C + b along the gaussian axis
(DVE tensor_tensor_scan, chained across 512-column chunks).

Culling: the image is split into 512 blocks of 8x16 pixels; a gaussian is
kept for a block only if its minimal sigma over the block (conservative
lambda_min eigenvalue bound) is < 21 (dropped alphas sum to < 1e-6).
This cuts the work ~7x. Blocks are snake-dealt by surviving-count rank
onto the 8 cores so every core gets an identical fixed slot schedule
(SPMD: one program, data-dependent content only). Each slot is padded at
the *front* with sentinel columns (beta=0 resets the scan state, b=0), so
every block's composite lands at a compile-time column.

Sharding: 8 NeuronCores; gaussian params replicated, blocks balanced;
host reassembles the image from the per-core slot outputs.
"""

import os
import numpy as np

H = 256
W = 256
N = 1024
NCORES = 8
BR, BC = 8, 16                 # block = 8 rows x 16 cols = 128 pixels
NBY, NBX = H // BR, W // BC
NBLK = NBY * NBX               # 512
SLOTS = NBLK // NCORES         # 64 slots per core
CULL_T = 21.0
SENT_NEG = -80.0
EPS2D = 0.3

_cache = {}


# ---------------------------------------------------------------- host math

def _preprocess(means, quats, scales, rgbs, opacities, viewmat, K):
    """Float64 per-gaussian preprocessing. Returns (in back-to-front order):
    G6 [6,N] basis coefficients of log(op)-sigma, colors [N],
    and (u, v, lam_min) for culling."""
    md = means.astype(np.float64)
    Rv = viewmat[:3, :3].astype(np.float64)
    t = viewmat[:3, 3].astype(np.float64)
    p_cam = md @ Rv.T + t
    x, y, z = p_cam[:, 0], p_cam[:, 1], p_cam[:, 2]
    fx, fy = float(K[0, 0]), float(K[1, 1])
    cx, cy = float(K[0, 2]), float(K[1, 2])
    inv_z = 1.0 / z
    u = fx * x * inv_z + cx
    v = fy * y * inv_z + cy

    th = quats.astype(np.float64)
    ct, st = np.cos(th), np.sin(th)
    zr, on = np.zeros_like(ct), np.ones_like(ct)
    R3 = np.stack([np.stack([ct, -st, zr], -1),
                   np.stack([st, ct, zr], -1),
                   np.stack([zr, zr, on], -1)], -2)
    M = R3 * scales.astype(np.float64)[:, None, :]
    cov3 = M @ np.swapaxes(M, -1, -2)
    cov_cam = np.einsum('ij,njk,lk->nil', Rv, cov3, Rv)
    j0 = np.stack([fx * inv_z, zr, -fx * x * inv_z * inv_z], -1)
    j1 = np.stack([zr, fy * inv_z, -fy * y * inv_z * inv_z], -1)
    J = np.stack([j0, j1], -2)
    cov2 = np.einsum('nij,njk,nlk->nil', J, cov_cam, J)
    a = cov2[:, 0, 0] + EPS2D
    b = cov2[:, 0, 1]
    c = cov2[:, 1, 1] + EPS2D
    det = a * c - b * b
    ca, cb, cc = c / det, -b / det, a / det

    op = 1.0 / (1.0 + np.exp(-opacities.astype(np.float64)))
    colv = 1.0 / (1.0 + np.exp(-rgbs.astype(np.float64)[:, 0]))

    # reference sorts by fp32 camera z ascending (stable); we composite
    # back-to-front = exact reverse
    order = np.argsort(z.astype(np.float32), kind="stable")
    rev = order[::-1]

    ca2, cc2 = 0.5 * ca, 0.5 * cc
    lop = np.log(op)
    d = -(ca * u + cb * v)
    e = -(cb * u + cc * v)
    f = ca2 * u * u + cb * u * v + cc2 * v * v
    G = np.stack([-ca2, -cb, -cc2, -d, -e, lop - f], 0)[:, rev]  # [6,N] f64
    colv = colv[rev]
    tr = ca + cc
    lam_min = 0.5 * (tr - np.sqrt((ca - cc) ** 2 + 4 * cb * cb))
    return G, colv, u[rev], v[rev], lam_min[rev]


def _build_schedule(G, colv, u, v, lam_min):
    """Cull per block, snake-deal blocks to cores, build the fixed slot
    schedule and the per-core gathered streams."""
    # exact minimal sigma over each block rectangle: 0 if the center is
    # inside, else the min over the four edges (1D quadratic, clamped)
    ca = -2.0 * G[0]
    cb = -G[1]
    cc = -2.0 * G[2]

    def sigma_at(dx, dy):
        return 0.5 * ca * dx * dx + cb * dx * dy + 0.5 * cc * dy * dy

    masks = np.zeros((NBLK, N), bool)
    for by in range(NBY):
        y0, y1 = by * BR + 0.5, by * BR + BR - 0.5
        for bx in range(NBX):
            x0, x1 = bx * BC + 0.5, bx * BC + BC - 0.5
            smin = np.full(N, np.inf)
            for xe in (x0, x1):
                dxe = xe - u
                dye = np.clip(-cb * dxe / cc, y0 - v, y1 - v)
                smin = np.minimum(smin, sigma_at(dxe, dye))
            for ye in (y0, y1):
                dye = ye - v
                dxe = np.clip(-cb * dye / ca, x0 - u, x1 - u)
                smin = np.minimum(smin, sigma_at(dxe, dye))
            inside = (u >= x0) & (u <= x1) & (v >= y0) & (v <= y1)
            smin[inside] = 0.0
            masks[by * NBX + bx] = smin < CULL_T
    widths = masks.sum(1)

    order = np.argsort(widths, kind="stable")[::-1]
    blk_of = np.zeros((NCORES, SLOTS), np.int32)
    for j in range(SLOTS):
        grp = order[j * NCORES:(j + 1) * NCORES]
        if j % 2 == 1:
            grp = grp[::-1]
        blk_of[:, j] = grp
    sched = widths[blk_of].max(0)
    slot_w = sched + 1                      # >=1 leading sentinel per slot
    ends = np.cumsum(slot_w)
    L = int(ends[-1])
    Lpad = (L + 511) // 512 * 512
    ends = ends + (Lpad - L)                # pad with sentinels at the start

    G6f = G.astype(np.float32)
    G6b = G6f.copy()
    G6b[5] = (G[5] + np.log(colv)).astype(np.float32)

    px = np.arange(W, dtype=np.float64) + 0.5
    py = np.arange(H, dtype=np.float64) + 0.5
    ft_blocks = np.zeros((NBLK, 6, 128), np.float32)
    for by in range(NBY):
        for bx in range(NBX):
            gy, gx = np.meshgrid(py[by * BR:(by + 1) * BR],
                                 px[bx * BC:(bx + 1) * BC], indexing="ij")
            fxr, fyr = gx.ravel(), gy.ravel()
            ft_blocks[by * NBX + bx] = np.stack(
                [fxr * fxr, fxr * fyr, fyr * fyr, fxr, fyr,
                 np.ones_like(fxr)], 0).astype(np.float32)

    cores = []
    for cid in range(NCORES):
        g1 = np.zeros((6, Lpad), np.float32)
        g2 = np.zeros((6, Lpad), np.float32)
        g2[5, :] = SENT_NEG
        colr = np.zeros(Lpad, np.float32)
        ft = np.zeros((6, SLOTS * 128), np.float32)
        for j in range(SLOTS):
            blk = blk_of[cid, j]
            idx = np.nonzero(masks[blk])[0]
            nb = len(idx)
            e0 = int(ends[j])
            g1[:, e0 - nb:e0] = G6f[:, idx]
            g2[:, e0 - nb:e0] = G6b[:, idx]
            colr[e0 - nb:e0] = colv[idx].astype(np.float32)
            ft[:, j * 128:(j + 1) * 128] = ft_blocks[blk]
        cores.append({"ft": ft, "g1": g1, "g2": g2, "colr": colr})
    return {"blk_of": blk_of, "ends": tuple(int(x) for x in ends),
            "Lpad": Lpad}, cores


# ---------------------------------------------------------------- device

def _build_module(ends, Lpad, reps=1, loop_n=1, variant="cb"):
    import contextlib
    import concourse.bass as bass
    import concourse.bacc as bacc
    import concourse.tile as tile
    from concourse import mybir

    f32 = mybir.dt.float32
    S = Lpad // 512

    # compile-time segmentation: slot ranges intersected with the 512 grid
    bounds = sorted(set(list(ends) + [k * 512 for k in range(S + 1)]))
    ends_arr = np.asarray(ends)
    segs = []                       # (slot, a, b)
    prev = 0
    for bnd in bounds:
        if bnd > prev:
            j = int(np.searchsorted(ends_arr, prev, side="right"))
            j = min(j, SLOTS - 1)   # leading global pad -> slot 0's lhsT
            segs.append((j, prev, bnd))
            prev = bnd
    chunk_segs = [[] for _ in range(S)]
    for j, a, b in segs:
        chunk_segs[a // 512].append((j, a, b))
    extract = [[] for _ in range(S)]   # (slot, offset-in-chunk)
    for j in range(SLOTS):
        pos = ends[j] - 1
        extract[pos // 512].append((j, pos % 512))

    nc = bacc.Bacc(None)
    ft = nc.dram_tensor("ft", [6, SLOTS * 128], f32, kind="ExternalInput")
    g1 = nc.dram_tensor("g1", [6, Lpad], f32, kind="ExternalInput")
    if variant == "mm2":
        g2 = nc.dram_tensor("g2", [6, Lpad], f32, kind="ExternalInput")
    else:
        colr = nc.dram_tensor("colr", [Lpad], f32, kind="ExternalInput")
    out = nc.dram_tensor("out", [128 * SLOTS], f32, kind="ExternalOutput")

    with tile.TileContext(nc) as tc:
        with (
            tc.tile_pool(name="const", bufs=1) as consts,
            tc.tile_pool(name="work", bufs=4) as work,
            tc.tile_pool(name="cpool", bufs=4) as cpool,
            tc.tile_pool(name="psum", bufs=6, space="PSUM") as psum,
        ):
            ft_s = consts.tile([6, SLOTS * 128], f32)
            nc.sync.dma_start(out=ft_s[:], in_=ft[:, :])
            g1_s = consts.tile([6, Lpad], f32)
            nc.sync.dma_start(out=g1_s[:], in_=g1[:, :])
            res = consts.tile([128, SLOTS], f32)
            if variant == "mm2":
                g2_s = consts.tile([6, Lpad], f32)
                nc.sync.dma_start(out=g2_s[:], in_=g2[:, :])
            else:
                c_s = consts.tile([128, Lpad], f32)
                step = Lpad // 8
                for q in range(8):
                    seg = colr[q * step:(q + 1) * step]
                    bc = bass.AP(tensor=seg.tensor, offset=seg.offset,
                                 ap=[[0, 128], seg.ap[0]])
                    nc.sync.dma_start(out=c_s[:, q * step:(q + 1) * step],
                                      in_=bc)

            loop_cm = (
                tc.For_i(0, loop_n, 1, hint_engines=(
                    mybir.EngineType.PE, mybir.EngineType.Activation,
                    mybir.EngineType.DVE))
                if loop_n > 1 else contextlib.nullcontext()
            )
            with loop_cm:
                for _ in range(reps):
                    prev_comp = None
                    ncopy = 0
                    for s in range(S):
                        m_ps = psum.tile(
                            [128, 1024 if variant == "mm2" else 512], f32)
                        for j, a, b in chunk_segs[s]:
                            lhs = ft_s[:, j * 128:(j + 1) * 128]
                            nc.tensor.matmul(
                                m_ps[:, a - s * 512:b - s * 512],
                                lhsT=lhs, rhs=g1_s[:, a:b],
                                start=True, stop=True,
                            )
                            if variant == "mm2":
                                nc.tensor.matmul(
                                    m_ps[:, 512 + a - s * 512:512 + b - s * 512],
                                    lhsT=lhs, rhs=g2_s[:, a:b],
                                    start=True, stop=True,
                                )
                        alpha = work.tile([128, 512], f32)
                        nc.scalar.activation(
                            out=alpha[:], in_=m_ps[:, 0:512],
                            func=mybir.ActivationFunctionType.Exp,
                            scale=1.0, bias=0.0,
                        )
                        bt = work.tile([128, 512], f32)
                        if variant == "mm2":
                            nc.scalar.activation(
                                out=bt[:], in_=m_ps[:, 512:1024],
                                func=mybir.ActivationFunctionType.Exp,
                                scale=1.0, bias=0.0,
                            )
                        else:
                            nc.vector.tensor_mul(
                                bt[:], alpha[:],
                                c_s[:, s * 512:(s + 1) * 512])
                        beta = work.tile([128, 512], f32)
                        nc.vector.tensor_scalar(
                            out=beta[:], in0=alpha[:], scalar1=-1.0, scalar2=1.0,
                            op0=mybir.AluOpType.mult, op1=mybir.AluOpType.add,
                        )
                        comp = cpool.tile([128, 512], f32)
                        init = 0.0 if prev_comp is None else prev_comp[:, 511:512]
                        nc.vector.tensor_tensor_scan(
                            comp[:], beta[:], bt[:], init,
                            op0=mybir.AluOpType.mult, op1=mybir.AluOpType.add,
                        )
                        prev_comp = comp
                        for j, off in extract[s]:
                            if ncopy % 2 == 0:
                                nc.scalar.copy(
                                    out=res[:, j:j + 1],
                                    in_=comp[:, off:off + 1])
                            else:
                                nc.vector.tensor_copy(
                                    res[:, j:j + 1], comp[:, off:off + 1])
                            ncopy += 1

                    nc.sync.dma_start(
                        out=out[:].rearrange("(k c) -> k c", c=SLOTS),
                        in_=res[:])
    nc.finalize()
    return nc


# ---------------------------------------------------------------- entry

def _prepare(inputs, reps=1, loop_n=1, variant=None):
    if variant is None:
        variant = os.environ.get("GS_VARIANT", "cb")
    G, colv, u, v, lam_min = _preprocess(**inputs)
    sched, cores = _build_schedule(G, colv, u, v, lam_min)
    key = (sched["ends"], sched["Lpad"], reps, loop_n, variant)
    if key not in _cache:
        _cache[key] = _build_module(
            sched["ends"], sched["Lpad"], reps=reps, loop_n=loop_n,
            variant=variant)
    nc = _cache[key]
    names = ("ft", "g1", "g2") if variant == "mm2" else ("ft", "g1", "colr")
    in_maps = [{k: cores[cid][k] for k in names} for cid in range(NCORES)]
    return nc, in_maps, sched


def _assemble(results, sched):
    img = np.zeros((H, W), np.float32)
    blk_of = sched["blk_of"]
    for cid in range(NCORES):
        res = results[cid]["out"].reshape(128, SLOTS)
        for j in range(SLOTS):
            by, bx = divmod(int(blk_of[cid, j]), NBX)
            img[by * BR:(by + 1) * BR, bx * BC:(bx + 1) * BC] = (
                res[:, j].reshape(BR, BC))
    return img.reshape(1, 1, H, W)


def kernel(**inputs):
    from concourse.bass_utils import run_bass_kernel_spmd

    inputs = {k: np.asarray(v) for k, v in inputs.items()}
    nc, in_maps, sched = _prepare(inputs)
    res = run_bass_kernel_spmd(nc, in_maps, core_ids=list(range(NCORES)))
    return _assemble(res.results, sched)



# revision 3
# speedup vs baseline: 2.0477x; 2.0477x over previous
"""Trainium2 Bass kernel for 2D Gaussian Splatting (N=1024 gaussians, 256x256).

Math: sigma[p,i] is a quadratic polynomial in pixel coords, so with a
block-centered pixel basis F[12,128] (6 monomials, each duplicated for a
hi/lo bf16 coefficient split) m1 = log(op) - sigma is ONE bf16 matmul
F.T @ G per 512-column chunk (the basis is block-independent in local
coords, so a single lhsT serves the whole stream). alpha = exp(m1) on
the scalar engine; a = alpha - 1 on gpsimd/DVE; compositing uses the
shifted state D = C - c (c = per-gaussian color) which obeys
    D_k = (delta_k - D_{k-1}) * a_k,   delta_k = c_k - c_{k-1},
one DVE tensor_tensor_scan (op0=subtract, op1=mult) per chunk with no
separate beta/bt tensors. delta is a host-precomputed constant. The host
adds back c at each slot's final column.

Culling: image split into 512 blocks of 8x16 pixels; a gaussian is kept
for a block iff its exact minimal sigma over the block is < 8 (dropped
alpha contributions < 2e-4 rel). Blocks are snake-dealt by surviving
count onto the 8 cores (SPMD: identical slot schedule, data-dependent
content only). Slots are front-padded with sentinel columns (all-zero
coefficients -> m1=0 -> a=0 resets the scan) to a multiple of 16
columns and bin-packed into independent 512-column chunks; only slots
wider than 512 chain scan state across their own consecutive chunks.
Slot composites are extracted with one strided copy (every 16th column)
+ one DMA; the host picks each slot's end column.

Sharding: 8 NeuronCores; gaussian params replicated, blocks balanced;
host reassembles the image from the per-core strided outputs.
"""

import numpy as np
import ml_dtypes

H = 256
W = 256
N = 1024
NCORES = 8
BR, BC = 8, 16                 # block = 8 rows x 16 cols = 128 pixels
NBY, NBX = H // BR, W // BC
NBLK = NBY * NBX               # 512
SLOTS = NBLK // NCORES         # 64 slots per core
CULL_T = 8.0
QU = 16                        # unit quantum (columns); slot ends at u*QU-1
CAP = 512 // QU                # units per chunk
EPS2D = 0.3

_cache = {}


# ---------------------------------------------------------------- host math

def _preprocess(means, quats, scales, rgbs, opacities, viewmat, K):
    """Float64 per-gaussian preprocessing. Returns, in back-to-front order:
    conic (ca, cb, cc), pixel means (u, v), log-opacity lop, colors colv."""
    md = means.astype(np.float64)
    Rv = viewmat[:3, :3].astype(np.float64)
    t = viewmat[:3, 3].astype(np.float64)
    p_cam = md @ Rv.T + t
    x, y, z = p_cam[:, 0], p_cam[:, 1], p_cam[:, 2]
    fx, fy = float(K[0, 0]), float(K[1, 1])
    cx, cy = float(K[0, 2]), float(K[1, 2])
    inv_z = 1.0 / z
    u = fx * x * inv_z + cx
    v = fy * y * inv_z + cy

    th = quats.astype(np.float64)
    ct, st = np.cos(th), np.sin(th)
    zr = np.zeros_like(ct)
    R3 = np.stack([np.stack([ct, -st, zr], -1),
                   np.stack([st, ct, zr], -1),
                   np.stack([zr, zr, np.ones_like(ct)], -1)], -2)
    M = R3 * scales.astype(np.float64)[:, None, :]
    cov3 = M @ np.swapaxes(M, -1, -2)
    cov_cam = np.einsum('ij,njk,lk->nil', Rv, cov3, Rv)
    j0 = np.stack([fx * inv_z, zr, -fx * x * inv_z * inv_z], -1)
    j1 = np.stack([zr, fy * inv_z, -fy * y * inv_z * inv_z], -1)
    J = np.stack([j0, j1], -2)
    cov2 = np.einsum('nij,njk,nlk->nil', J, cov_cam, J)
    a = cov2[:, 0, 0] + EPS2D
    b = cov2[:, 0, 1]
    c = cov2[:, 1, 1] + EPS2D
    det = a * c - b * b
    ca, cb, cc = c / det, -b / det, a / det

    op = 1.0 / (1.0 + np.exp(-opacities.astype(np.float64)))
    colv = 1.0 / (1.0 + np.exp(-rgbs.astype(np.float64)[:, 0]))

    # reference sorts by fp32 camera z ascending (stable); we composite
    # back-to-front = exact reverse
    order = np.argsort(z.astype(np.float32), kind="stable")
    rev = order[::-1]
    return (ca[rev], cb[rev], cc[rev], u[rev], v[rev],
            np.log(op)[rev], colv[rev])


def _block_masks(ca, cb, cc, u, v):
    """Exact minimal sigma over each block rectangle: 0 if the center is
    inside, else the min over the four edges (1D quadratic, clamped)."""
    def sigma_at(dx, dy):
        return 0.5 * ca * dx * dx + cb * dx * dy + 0.5 * cc * dy * dy

    masks = np.zeros((NBLK, N), bool)
    for by in range(NBY):
        y0, y1 = by * BR + 0.5, by * BR + BR - 0.5
        for bx in range(NBX):
            x0, x1 = bx * BC + 0.5, bx * BC + BC - 0.5
            smin = np.full(N, np.inf)
            for xe in (x0, x1):
                dxe = xe - u
                dye = np.clip(-cb * dxe / cc, y0 - v, y1 - v)
                smin = np.minimum(smin, sigma_at(dxe, dye))
            for ye in (y0, y1):
                dye = ye - v
                dxe = np.clip(-cb * dye / ca, x0 - u, x1 - u)
                smin = np.minimum(smin, sigma_at(dxe, dye))
            inside = (u >= x0) & (u <= x1) & (v >= y0) & (v <= y1)
            smin[inside] = 0.0
            masks[by * NBX + bx] = smin < CULL_T
    return masks


def _local_basis():
    """[12, 128] block-local monomial basis (exact in bf16)."""
    px = np.arange(BC) + 0.5 - BC / 2.0          # -7.5 .. 7.5
    py = np.arange(BR) + 0.5 - BR / 2.0          # -3.5 .. 3.5
    gx, gy = np.meshgrid(px, py)                 # [BR, BC] row-major
    fx, fy = gx.ravel(), gy.ravel()
    rows = [fx * fx, fx * fy, fy * fy, fx, fy, np.ones_like(fx)]
    return np.repeat(np.stack(rows, 0), 2, axis=0)  # each row duplicated


def _build_schedule(ca, cb, cc, u, v, lop, colv):
    """Cull per block, snake-deal blocks to cores, bin-pack padded slots
    into independent 512-col chunks, build per-core device arrays."""
    masks = _block_masks(ca, cb, cc, u, v)
    widths = masks.sum(1)

    order = np.argsort(widths, kind="stable")[::-1]
    blk_of = np.zeros((NCORES, SLOTS), np.int32)
    for j in range(SLOTS):
        grp = order[j * NCORES:(j + 1) * NCORES]
        if j % 2 == 1:
            grp = grp[::-1]
        blk_of[:, j] = grp
    slot_w = widths[blk_of].max(0)                       # shared schedule
    units = (slot_w + 1 + QU - 1) // QU                  # >=1 sentinel col

    # --- pack: oversize slots get dedicated consecutive chunks (scan
    # carries across them); regular slots first-fit-decreasing into bins
    desc = sorted(range(SLOTS), key=lambda j: -units[j])
    place = np.zeros(SLOTS, np.int64)                    # absolute unit start
    carry = []                                           # per chunk
    abs_u = 0
    regular = []
    for j in desc:
        uj = int(units[j])
        if uj > CAP:
            run = -(-uj // CAP)
            place[j] = abs_u + run * CAP - uj
            carry.extend([False] + [True] * (run - 1))
            abs_u += run * CAP
        else:
            regular.append(j)
    fills, members = [], []                              # per bin
    for j in regular:
        uj = int(units[j])
        for i in range(len(fills)):
            if fills[i] + uj <= CAP:
                place[j] = fills[i]                      # offset; base later
                members[i].append(j)
                fills[i] += uj
                break
        else:
            place[j] = 0
            members.append([j])
            fills.append(uj)
    for i in range(len(fills)):
        for j in members[i]:
            place[j] += abs_u + i * CAP
        carry.append(False)
    abs_u += len(fills) * CAP
    Lpad = abs_u * QU
    S = Lpad // 512
    assert len(carry) == S

    bf16 = ml_dtypes.bfloat16
    cores = []
    c_last = np.zeros((NCORES, SLOTS), np.float64)
    for cid in range(NCORES):
        g12 = np.zeros((12, Lpad), np.float64)
        delta = np.zeros(Lpad, np.float32)
        for j in range(SLOTS):
            blk = int(blk_of[cid, j])
            idx = np.nonzero(masks[blk])[0]
            nb = len(idx)
            if nb == 0:
                continue
            end = (int(place[j]) + int(units[j])) * QU
            s0 = end - nb
            by, bx = divmod(blk, NBX)
            cxb = bx * BC + BC / 2.0
            cyb = by * BR + BR / 2.0
            du = u[idx] - cxb
            dv = v[idx] - cyb
            cai, cbi, cci = ca[idx], cb[idx], cc[idx]
            gs = [-0.5 * cai, -cbi, -0.5 * cci,
                  cai * du + cbi * dv, cbi * du + cci * dv,
                  lop[idx] - (0.5 * cai * du * du + cbi * du * dv
                              + 0.5 * cci * dv * dv)]
            for r, g in enumerate(gs):
                hi = np.asarray(g, dtype=bf16).astype(np.float64)
                lo = g - hi
                g12[2 * r, s0:end] = hi
                g12[2 * r + 1, s0:end] = lo
            cv = colv[idx]
            delta[s0:end] = (cv - np.concatenate([[0.0], cv[:-1]])
                             ).astype(np.float32)
            c_last[cid, j] = cv[-1]
        cores.append({"g12": g12.astype(bf16), "delta": delta})
    sched = {"blk_of": blk_of, "place": place, "units": units,
             "carry": tuple(carry), "Lpad": int(Lpad), "c_last": c_last}
    return sched, cores


# ---------------------------------------------------------------- device

def _build_module(Lpad, carry, reps=1, loop_n=1):
    import contextlib
    import concourse.bass as bass
    import concourse.bacc as bacc
    import concourse.tile as tile
    from concourse import mybir

    f32 = mybir.dt.float32
    bf16 = mybir.dt.bfloat16
    S = Lpad // 512
    U = Lpad // QU

    nc = bacc.Bacc(None)
    ft = nc.dram_tensor("ft", [12, 128], bf16, kind="ExternalInput")
    g12 = nc.dram_tensor("g12", [12, Lpad], bf16, kind="ExternalInput")
    delta = nc.dram_tensor("delta", [Lpad], f32, kind="ExternalInput")
    out = nc.dram_tensor("out", [128 * U], f32, kind="ExternalOutput")

    with tile.TileContext(nc) as tc:
        with (
            tc.tile_pool(name="const", bufs=1) as consts,
            tc.tile_pool(name="work", bufs=6) as work,
            tc.tile_pool(name="psum", bufs=8, space="PSUM") as psum,
        ):
            ft_s = consts.tile([12, 128], bf16)
            nc.sync.dma_start(out=ft_s[:], in_=ft[:, :])
            g_s = consts.tile([12, Lpad], bf16)
            nc.sync.dma_start(out=g_s[:], in_=g12[:, :])
            d_s = consts.tile([128, Lpad], f32)
            step = Lpad // 8
            for q in range(8):
                seg = delta[q * step:(q + 1) * step]
                bc = bass.AP(tensor=seg.tensor, offset=seg.offset,
                             ap=[[0, 128], seg.ap[0]])
                nc.sync.dma_start(out=d_s[:, q * step:(q + 1) * step], in_=bc)
            D = consts.tile([128, Lpad], f32)
            res = consts.tile([128, U], f32)

            loop_cm = (
                tc.For_i(0, loop_n, 1, hint_engines=(
                    mybir.EngineType.PE, mybir.EngineType.Activation,
                    mybir.EngineType.DVE, mybir.EngineType.Pool))
                if loop_n > 1 else contextlib.nullcontext()
            )
            with loop_cm:
                for _ in range(reps):
                    for s in range(S):
                        sl = slice(s * 512, (s + 1) * 512)
                        m_ps = psum.tile([128, 512], f32)
                        nc.tensor.matmul(
                            m_ps[:, :], lhsT=ft_s[:, :], rhs=g_s[:, sl],
                            start=True, stop=True,
                        )
                        a_t = work.tile([128, 512], f32)
                        nc.scalar.activation(
                            out=a_t[:], in_=m_ps[:, :],
                            func=mybir.ActivationFunctionType.Exp,
                            scale=1.0, bias=0.0,
                        )
                        am_t = work.tile([128, 512], f32)
                        eng = nc.gpsimd if s % 2 == 0 else nc.vector
                        eng.tensor_scalar_add(am_t[:], a_t[:], -1.0)
                        init = (D[:, s * 512 - 1:s * 512] if carry[s]
                                else 0.0)
                        nc.vector.tensor_tensor_scan(
                            D[:, sl], d_s[:, sl], am_t[:], init,
                            op0=mybir.AluOpType.subtract,
                            op1=mybir.AluOpType.mult,
                        )
                    dfull = D[:]
                    strided = bass.AP(
                        tensor=dfull.tensor, offset=dfull.offset + (QU - 1),
                        ap=[dfull.ap[0], [QU, U]])
                    nc.scalar.copy(out=res[:], in_=strided)
                    nc.sync.dma_start(
                        out=out[:].rearrange("(k c) -> k c", c=U),
                        in_=res[:])
    nc.finalize()
    return nc


# ---------------------------------------------------------------- entry

def _prepare(inputs, reps=1, loop_n=1, variant=None):
    ca, cb, cc, u, v, lop, colv = _preprocess(**inputs)
    sched, cores = _build_schedule(ca, cb, cc, u, v, lop, colv)
    key = (sched["carry"], sched["Lpad"], reps, loop_n)
    if key not in _cache:
        _cache[key] = _build_module(
            sched["Lpad"], sched["carry"], reps=reps, loop_n=loop_n)
    nc = _cache[key]
    ftb = _local_basis().astype(ml_dtypes.bfloat16)
    in_maps = [{"ft": ftb, "g12": cores[cid]["g12"],
                "delta": cores[cid]["delta"]} for cid in range(NCORES)]
    return nc, in_maps, sched


def _assemble(results, sched):
    img = np.zeros((H, W), np.float32)
    blk_of = sched["blk_of"]
    place, units = sched["place"], sched["units"]
    U = sched["Lpad"] // QU
    for cid in range(NCORES):
        res = results[cid]["out"].reshape(128, U)
        for j in range(SLOTS):
            by, bx = divmod(int(blk_of[cid, j]), NBX)
            uend = int(place[j]) + int(units[j]) - 1
            col = res[:, uend] + np.float32(sched["c_last"][cid, j])
            img[by * BR:(by + 1) * BR, bx * BC:(bx + 1) * BC] = (
                col.reshape(BR, BC))
    return img.reshape(1, 1, H, W)


def kernel(**inputs):
    from concourse.bass_utils import run_bass_kernel_spmd

    inputs = {k: np.asarray(v) for k, v in inputs.items()}
    nc, in_maps, sched = _prepare(inputs)
    res = run_bass_kernel_spmd(nc, in_maps, core_ids=list(range(NCORES)))
    return _assemble(res.results, sched)


# revision 6
# speedup vs baseline: 9.9207x; 4.8448x over previous
"""Trainium2 Bass kernel for 2D Gaussian Splatting (N=1024 gaussians, 256x256).

Math: sigma[p,i] is a quadratic polynomial in pixel coords, so with a
block-centered pixel basis F[12,128] (6 monomials, each duplicated for a
hi/lo bf16 coefficient split) m1 = log(op) - sigma is ONE bf16 matmul
F.T @ G per 512-column chunk (the basis is block-independent in local
coords, so a single lhsT serves the whole stream). alpha = exp(m1) on
the scalar engine; a = alpha - 1 on gpsimd/DVE; compositing uses the
shifted state D = C - c (c = per-gaussian color) which obeys
    D_k = (delta_k - D_{k-1}) * a_k,   delta_k = c_k - c_{k-1},
one DVE tensor_tensor_scan (op0=subtract, op1=mult) per chunk with no
separate beta/bt tensors. delta is a host-precomputed constant. The host
adds back c at each slot's final column.

Culling: image split into 512 blocks of 8x16 pixels; a gaussian is kept
for a block iff its exact minimal sigma over the block is < 8 (dropped
alpha contributions < 2e-4 rel). Blocks are snake-dealt by surviving
count onto the 8 cores (SPMD: identical slot schedule, data-dependent
content only). Slots are front-padded with sentinel columns (all-zero
coefficients -> m1=0 -> a=0 resets the scan) to a multiple of 16
columns and bin-packed into independent 512-column chunks; only slots
wider than 512 chain scan state across their own consecutive chunks.
Slot composites are extracted with one strided copy (every 16th column)
+ one DMA; the host picks each slot's end column.

Sharding: 8 NeuronCores; gaussian params replicated, blocks balanced;
host reassembles the image from the per-core strided outputs.
"""

import numpy as np
import ml_dtypes

H = 256
W = 256
N = 1024
NCORES = 8
BR, BC = 8, 16                 # block = 8 rows x 16 cols = 128 pixels
NBY, NBX = H // BR, W // BC
NBLK = NBY * NBX               # 512
SLOTS = NBLK // NCORES         # 64 slots per core
CULL_T = 8.0
QU = 16                        # unit quantum (columns); slot ends at u*QU-1
CAP = 512 // QU                # units per chunk
EPS2D = 0.3

_cache = {}


# ---------------------------------------------------------------- host math

def _preprocess(means, quats, scales, rgbs, opacities, viewmat, K):
    """Float64 per-gaussian preprocessing. Returns, in back-to-front order:
    conic (ca, cb, cc), pixel means (u, v), log-opacity lop, colors colv."""
    md = means.astype(np.float64)
    Rv = viewmat[:3, :3].astype(np.float64)
    t = viewmat[:3, 3].astype(np.float64)
    p_cam = md @ Rv.T + t
    x, y, z = p_cam[:, 0], p_cam[:, 1], p_cam[:, 2]
    fx, fy = float(K[0, 0]), float(K[1, 1])
    cx, cy = float(K[0, 2]), float(K[1, 2])
    inv_z = 1.0 / z
    u = fx * x * inv_z + cx
    v = fy * y * inv_z + cy

    th = quats.astype(np.float64)
    ct, st = np.cos(th), np.sin(th)
    zr = np.zeros_like(ct)
    R3 = np.stack([np.stack([ct, -st, zr], -1),
                   np.stack([st, ct, zr], -1),
                   np.stack([zr, zr, np.ones_like(ct)], -1)], -2)
    M = R3 * scales.astype(np.float64)[:, None, :]
    cov3 = M @ np.swapaxes(M, -1, -2)
    cov_cam = np.einsum('ij,njk,lk->nil', Rv, cov3, Rv)
    j0 = np.stack([fx * inv_z, zr, -fx * x * inv_z * inv_z], -1)
    j1 = np.stack([zr, fy * inv_z, -fy * y * inv_z * inv_z], -1)
    J = np.stack([j0, j1], -2)
    cov2 = np.einsum('nij,njk,nlk->nil', J, cov_cam, J)
    a = cov2[:, 0, 0] + EPS2D
    b = cov2[:, 0, 1]
    c = cov2[:, 1, 1] + EPS2D
    det = a * c - b * b
    ca, cb, cc = c / det, -b / det, a / det

    op = 1.0 / (1.0 + np.exp(-opacities.astype(np.float64)))
    colv = 1.0 / (1.0 + np.exp(-rgbs.astype(np.float64)[:, 0]))

    # reference sorts by fp32 camera z ascending (stable); we composite
    # back-to-front = exact reverse
    order = np.argsort(z.astype(np.float32), kind="stable")
    rev = order[::-1]
    return (ca[rev], cb[rev], cc[rev], u[rev], v[rev],
            np.log(op)[rev], colv[rev])


def _block_masks(ca, cb, cc, u, v):
    """Exact minimal sigma over each block rectangle: 0 if the center is
    inside, else the min over the four edges (1D quadratic, clamped)."""
    def sigma_at(dx, dy):
        return 0.5 * ca * dx * dx + cb * dx * dy + 0.5 * cc * dy * dy

    masks = np.zeros((NBLK, N), bool)
    for by in range(NBY):
        y0, y1 = by * BR + 0.5, by * BR + BR - 0.5
        for bx in range(NBX):
            x0, x1 = bx * BC + 0.5, bx * BC + BC - 0.5
            smin = np.full(N, np.inf)
            for xe in (x0, x1):
                dxe = xe - u
                dye = np.clip(-cb * dxe / cc, y0 - v, y1 - v)
                smin = np.minimum(smin, sigma_at(dxe, dye))
            for ye in (y0, y1):
                dye = ye - v
                dxe = np.clip(-cb * dye / ca, x0 - u, x1 - u)
                smin = np.minimum(smin, sigma_at(dxe, dye))
            inside = (u >= x0) & (u <= x1) & (v >= y0) & (v <= y1)
            smin[inside] = 0.0
            masks[by * NBX + bx] = smin < CULL_T
    return masks


def _local_basis():
    """[12, 128] block-local monomial basis (exact in bf16)."""
    px = np.arange(BC) + 0.5 - BC / 2.0          # -7.5 .. 7.5
    py = np.arange(BR) + 0.5 - BR / 2.0          # -3.5 .. 3.5
    gx, gy = np.meshgrid(px, py)                 # [BR, BC] row-major
    fx, fy = gx.ravel(), gy.ravel()
    rows = [fx * fx, fx * fy, fy * fy, fx, fy, np.ones_like(fx)]
    return np.repeat(np.stack(rows, 0), 2, axis=0)  # each row duplicated


def _build_schedule(ca, cb, cc, u, v, lop, colv):
    """Cull per block, snake-deal blocks to cores, bin-pack padded slots
    into independent 512-col chunks, build per-core device arrays."""
    masks = _block_masks(ca, cb, cc, u, v)
    widths = masks.sum(1)

    order = np.argsort(widths, kind="stable")[::-1]
    blk_of = np.zeros((NCORES, SLOTS), np.int32)
    for j in range(SLOTS):
        grp = order[j * NCORES:(j + 1) * NCORES]
        if j % 2 == 1:
            grp = grp[::-1]
        blk_of[:, j] = grp
    slot_w = widths[blk_of].max(0)                       # shared schedule
    units = (slot_w + 1 + QU - 1) // QU                  # >=1 sentinel col

    # --- pack: oversize slots get dedicated consecutive chunks (scan
    # carries across them); regular slots first-fit-decreasing into bins
    desc = sorted(range(SLOTS), key=lambda j: -units[j])
    place = np.zeros(SLOTS, np.int64)                    # absolute unit start
    carry = []                                           # per chunk
    abs_u = 0
    regular = []
    for j in desc:
        uj = int(units[j])
        if uj > CAP:
            run = -(-uj // CAP)
            place[j] = abs_u + run * CAP - uj
            carry.extend([False] + [True] * (run - 1))
            abs_u += run * CAP
        else:
            regular.append(j)
    fills, members = [], []                              # per bin
    for j in regular:
        uj = int(units[j])
        for i in range(len(fills)):
            if fills[i] + uj <= CAP:
                place[j] = fills[i]                      # offset; base later
                members[i].append(j)
                fills[i] += uj
                break
        else:
            place[j] = 0
            members.append([j])
            fills.append(uj)
    for i in range(len(fills)):
        for j in members[i]:
            place[j] += abs_u + i * CAP
        carry.append(False)
    abs_u += len(fills) * CAP
    Lpad = abs_u * QU
    S = Lpad // 512
    assert len(carry) == S

    bf16 = ml_dtypes.bfloat16
    cores = []
    c_last = np.zeros((NCORES, SLOTS), np.float64)
    for cid in range(NCORES):
        g12 = np.zeros((12, Lpad), np.float64)
        delta = np.zeros(Lpad, np.float32)
        for j in range(SLOTS):
            blk = int(blk_of[cid, j])
            idx = np.nonzero(masks[blk])[0]
            nb = len(idx)
            if nb == 0:
                continue
            end = (int(place[j]) + int(units[j])) * QU
            s0 = end - nb
            by, bx = divmod(blk, NBX)
            cxb = bx * BC + BC / 2.0
            cyb = by * BR + BR / 2.0
            du = u[idx] - cxb
            dv = v[idx] - cyb
            cai, cbi, cci = ca[idx], cb[idx], cc[idx]
            gs = [-0.5 * cai, -cbi, -0.5 * cci,
                  cai * du + cbi * dv, cbi * du + cci * dv,
                  lop[idx] - (0.5 * cai * du * du + cbi * du * dv
                              + 0.5 * cci * dv * dv)]
            for r, g in enumerate(gs):
                hi = np.asarray(g, dtype=bf16).astype(np.float64)
                lo = g - hi
                g12[2 * r, s0:end] = hi
                g12[2 * r + 1, s0:end] = lo
            cv = colv[idx]
            delta[s0:end] = (cv - np.concatenate([[0.0], cv[:-1]])
                             ).astype(np.float32)
            c_last[cid, j] = cv[-1]
        cores.append({"g12": g12.astype(bf16), "delta": delta})
    sched = {"blk_of": blk_of, "place": place, "units": units,
             "carry": tuple(carry), "Lpad": int(Lpad), "c_last": c_last}
    return sched, cores


# ---------------------------------------------------------------- device

def _build_module(Lpad, carry, reps=1, loop_n=1, stagger=True):
    import contextlib
    import concourse.bass as bass
    import concourse.bacc as bacc
    import concourse.tile as tile
    from concourse import mybir

    f32 = mybir.dt.float32
    bf16 = mybir.dt.bfloat16
    S = Lpad // 512
    U = Lpad // QU

    nc = bacc.Bacc(None)
    ft = nc.dram_tensor("ft", [12, 128], bf16, kind="ExternalInput")
    g12 = nc.dram_tensor("g12", [12, Lpad], bf16, kind="ExternalInput")
    delta = nc.dram_tensor("delta", [Lpad], f32, kind="ExternalInput")
    out = nc.dram_tensor("out", [128 * U], f32, kind="ExternalOutput")

    with tile.TileContext(nc) as tc:
        with (
            tc.tile_pool(name="const", bufs=1) as consts,
            tc.tile_pool(name="work", bufs=6) as work,
            tc.tile_pool(name="psum", bufs=8, space="PSUM") as psum,
        ):
            ft_s = consts.tile([12, 128], bf16)
            nc.sync.dma_start(out=ft_s[:], in_=ft[:, :])
            g_s = consts.tile([12, Lpad], bf16)
            nc.sync.dma_start(out=g_s[:], in_=g12[:, :])
            d_s = consts.tile([128, Lpad], f32)
            step = Lpad // 8
            for q in range(8):
                seg = delta[q * step:(q + 1) * step]
                bc = bass.AP(tensor=seg.tensor, offset=seg.offset,
                             ap=[[0, 128], seg.ap[0]])
                nc.sync.dma_start(out=d_s[:, q * step:(q + 1) * step], in_=bc)
            D = consts.tile([128, Lpad], f32)
            res = consts.tile([128, U], f32)

            loop_cm = (
                tc.For_i(0, loop_n, 1, hint_engines=(
                    mybir.EngineType.PE, mybir.EngineType.Activation,
                    mybir.EngineType.DVE, mybir.EngineType.Pool),
                    staggered_reset=stagger)
                if loop_n > 1 else contextlib.nullcontext()
            )
            with loop_cm:
                for _ in range(reps):
                    for s in range(S):
                        sl = slice(s * 512, (s + 1) * 512)
                        m_ps = psum.tile([128, 512], f32)
                        nc.tensor.matmul(
                            m_ps[:, :], lhsT=ft_s[:, :], rhs=g_s[:, sl],
                            start=True, stop=True,
                        )
                        a_t = work.tile([128, 512], f32)
                        nc.scalar.activation(
                            out=a_t[:], in_=m_ps[:, :],
                            func=mybir.ActivationFunctionType.Exp,
                            scale=1.0, bias=0.0,
                        )
                        am_t = work.tile([128, 512], f32)
                        eng = nc.gpsimd if s % 2 == 0 else nc.vector
                        eng.tensor_scalar_add(am_t[:], a_t[:], -1.0)
                        init = (D[:, s * 512 - 1:s * 512] if carry[s]
                                else 0.0)
                        nc.vector.tensor_tensor_scan(
                            D[:, sl], d_s[:, sl], am_t[:], init,
                            op0=mybir.AluOpType.subtract,
                            op1=mybir.AluOpType.mult,
                        )
                    dfull = D[:]
                    strided = bass.AP(
                        tensor=dfull.tensor, offset=dfull.offset + (QU - 1),
                        ap=[dfull.ap[0], [QU, U]])
                    nc.scalar.copy(out=res[:], in_=strided)
                    nc.sync.dma_start(
                        out=out[:].rearrange("(k c) -> k c", c=U),
                        in_=res[:])
    nc.finalize()
    return nc


# ---------------------------------------------------------------- entry

def _prepare(inputs, reps=1, loop_n=1, variant=None, stagger=True):
    ca, cb, cc, u, v, lop, colv = _preprocess(**inputs)
    sched, cores = _build_schedule(ca, cb, cc, u, v, lop, colv)
    key = (sched["carry"], sched["Lpad"], reps, loop_n, stagger)
    if key not in _cache:
        _cache[key] = _build_module(
            sched["Lpad"], sched["carry"], reps=reps, loop_n=loop_n,
            stagger=stagger)
    nc = _cache[key]
    ftb = _local_basis().astype(ml_dtypes.bfloat16)
    in_maps = [{"ft": ftb, "g12": cores[cid]["g12"],
                "delta": cores[cid]["delta"]} for cid in range(NCORES)]
    return nc, in_maps, sched


def _assemble(results, sched):
    img = np.zeros((H, W), np.float32)
    blk_of = sched["blk_of"]
    place, units = sched["place"], sched["units"]
    U = sched["Lpad"] // QU
    for cid in range(NCORES):
        res = results[cid]["out"].reshape(128, U)
        for j in range(SLOTS):
            by, bx = divmod(int(blk_of[cid, j]), NBX)
            uend = int(place[j]) + int(units[j]) - 1
            col = res[:, uend] + np.float32(sched["c_last"][cid, j])
            img[by * BR:(by + 1) * BR, bx * BC:(bx + 1) * BC] = (
                col.reshape(BR, BC))
    return img.reshape(1, 1, H, W)


def kernel(**inputs):
    from concourse.bass_utils import run_bass_kernel_spmd

    inputs = {k: np.asarray(v) for k, v in inputs.items()}
    nc, in_maps, sched = _prepare(inputs)
    res = run_bass_kernel_spmd(nc, in_maps, core_ids=list(range(NCORES)))
    return _assemble(res.results, sched)
